# revision 16
# baseline (speedup 1.0000x reference)
"""Causal single-head attention (shared-weight multi-head), 8-core Trainium2 Bass kernel.

Problem: embedded [4, 4096, 1024] f32, Wq/Wk/Wv [1024, 64] f32.
  q/k/v = embedded @ W*;  S = q k^T / 8 (causal);  P = softmax(S);  head = P v
  output = tile(head, 16) -> [4, 4096, 1024] f32.

Sharding: 8 cores = 4 batches x 2 roles. Per batch the 4096 rows form 8 blocks
of 512; role 0 owns blocks {0,3,4,7}, role 1 owns {1,2,5,6} (causal work per
core is equal: sum(j+1) = 18 for both). Every core runs the SAME program over
4 "slots" (one per owned block, ascending); slot s processes a padded causal
extent of {1024,2048,3072,4096} columns. The host permutes each core's
transposed input so owned blocks sit first (positions 0-3) followed by the
other role's blocks (positions 4-7); causality then maps to a role-independent
set of column blocks per slot, with one potentially-padded column block per
slot whose validity is passed as per-core 0/1 data (padmask).

On-chip math (all matmuls bf16 with f32 PSUM accumulation):
  K^T [64, 4096] and Q^T [64, 2048] and V [4096, 64] projections
  S^T tile [128 cols, 512 rows] = (K^T chunk)^T-contract Q^T   (K=64 matmul)
  P~ = exp(S/8) via ACT (scores are bounded ~2.6, so no max subtraction),
       cast to bf16; causal tri-mask on the diagonal block, padmask on the
       padded block
  out [512 rows, 65] accumulated over col chunks with rhs = [V | 1]; column 64
       gives the softmax denominator for free. Divide via DVE reciprocal, then
       replicate x16 along features and DMA out.
"""

import os
import numpy as np
import ml_dtypes

B, T, E, HEAD, NH = 4, 4096, 1024, 64, 16
BLK = 512
NCORES = 8
OWN = {0: [0, 3, 4, 7], 1: [1, 2, 5, 6]}
PADS = [1, 2, 3, 4]  # padded count of "other role" 512-blocks visible per slot
PADMASK = {0: [0.0, 1.0, 0.0, 1.0], 1: [1.0, 0.0, 1.0, 0.0]}

_prog_cache = {}


def _build_program(reps=None):
    import concourse.bass as bass
    import concourse.mybir as mybir
    import concourse.tile as tile
    from concourse import bacc

    f32 = mybir.dt.float32
    bf16 = mybir.dt.bfloat16

    nc = bacc.Bacc("TRN2", target_bir_lowering=False, debug=False, num_devices=NCORES)

    xT = nc.dram_tensor("xT", [E, T], bf16, kind="ExternalInput").ap()
    wq = nc.dram_tensor("wq", [E, HEAD], bf16, kind="ExternalInput").ap()
    wk = nc.dram_tensor("wk", [E, HEAD], bf16, kind="ExternalInput").ap()
    wv = nc.dram_tensor("wv", [E, HEAD], bf16, kind="ExternalInput").ap()
    tri = nc.dram_tensor("tri", [128, 4, BLK], bf16, kind="ExternalInput").ap()
    padmask = nc.dram_tensor("padmask", [128, 4], f32, kind="ExternalInput").ap()
    out = nc.dram_tensor("out", [128, 16, HEAD], f32, kind="ExternalOutput").ap()

    KE = E // 128  # contraction chunks for projections

    import contextlib

    with tile.TileContext(nc) as tc:
        loop_ctx = tc.For_i(0, reps, 1) if reps else contextlib.nullcontext()
        with (
            loop_ctx,
            tc.tile_pool(name="singles", bufs=1) as singles,
            tc.tile_pool(name="psum_proj", bufs=2, space="PSUM") as psum_proj,
            tc.tile_pool(name="psum_s", bufs=2, space="PSUM") as psum_s,
            tc.tile_pool(name="psum_o", bufs=4, space="PSUM") as psum_o,
            tc.tile_pool(name="ptil", bufs=8) as ptil_pool,
            tc.tile_pool(name="work", bufs=4) as work,
        ):
            # ---- load inputs ----
            x_sb = singles.tile([128, KE, T], bf16)
            for k in range(KE):
                nc.sync.dma_start(out=x_sb[:, k, :], in_=xT[k * 128:(k + 1) * 128, :])
            wq_sb = singles.tile([128, KE, HEAD], bf16)
            wk_sb = singles.tile([128, KE, HEAD], bf16)
            wv_sb = singles.tile([128, KE, HEAD], bf16)
            nc.sync.dma_start(out=wq_sb, in_=wq.rearrange("(k p) d -> p k d", p=128))
            nc.sync.dma_start(out=wk_sb, in_=wk.rearrange("(k p) d -> p k d", p=128))
            nc.sync.dma_start(out=wv_sb, in_=wv.rearrange("(k p) d -> p k d", p=128))
            tri_sb = singles.tile([128, 4, BLK], bf16)
            nc.sync.dma_start(out=tri_sb, in_=tri)
            pm_sb = singles.tile([128, 4], f32)
            nc.sync.dma_start(out=pm_sb, in_=padmask)

            # ---- projections ----
            KT = singles.tile([64, T], bf16)       # K^T for all 8 blocks
            QT = singles.tile([64, 4 * BLK], bf16)  # Q^T for own blocks (cols 0-2047)
            V1 = singles.tile([128, T // 128, HEAD + 1], bf16)  # [V | 1] per col chunk

            for blk in range(T // BLK):
                ps = psum_proj.tile([64, BLK], f32, tag="proj")
                for k in range(KE):
                    nc.tensor.matmul(
                        ps, wk_sb[:, k, :], x_sb[:, k, blk * BLK:(blk + 1) * BLK],
                        start=(k == 0), stop=(k == KE - 1),
                    )
                nc.vector.tensor_copy(KT[:, blk * BLK:(blk + 1) * BLK], ps)
            for blk in range(4):
                ps = psum_proj.tile([64, BLK], f32, tag="proj")
                for k in range(KE):
                    nc.tensor.matmul(
                        ps, wq_sb[:, k, :], x_sb[:, k, blk * BLK:(blk + 1) * BLK],
                        start=(k == 0), stop=(k == KE - 1),
                    )
                nc.vector.tensor_copy(QT[:, blk * BLK:(blk + 1) * BLK], ps)
            for g in range(T // 128):
                ps = psum_proj.tile([128, HEAD], f32, tag="proj")
                for k in range(KE):
                    nc.tensor.matmul(
                        ps, x_sb[:, k, g * 128:(g + 1) * 128], wv_sb[:, k, :],
                        start=(k == 0), stop=(k == KE - 1),
                    )
                nc.vector.tensor_copy(V1[:, g, 0:HEAD], ps)
                nc.vector.memset(V1[:, g, HEAD:HEAD + 1], 1.0)

            outs_sb = singles.tile([128, 16, HEAD], f32)
            # ---- attention, one slot per owned block ----
            for s in range(4):
                own_chunks = 4 * (s + 1)          # 128-col chunks in own region
                other_chunks = 4 * PADS[s]        # 128-col chunks in other region
                globs = list(range(own_chunks)) + [
                    16 + c for c in range(other_chunks)
                ]
                nC = len(globs)
                # one PSUM bank per 128-row chunk: accumulation groups must
                # not share a bank (start=True clears the bank zero-region)
                o_tiles = [psum_o.tile([128, HEAD + 1], f32, tag="o",
                                       name=f"o_s{s}r{r}")
                           for r in range(4)]
                for ci, g in enumerate(globs):
                    s_ps = psum_s.tile([128, BLK], f32, tag="s")
                    nc.tensor.matmul(
                        s_ps, KT[:, g * 128:(g + 1) * 128],
                        QT[:, s * BLK:(s + 1) * BLK],
                        start=True, stop=True,
                    )
                    pt = ptil_pool.tile([128, BLK], bf16, tag="pt")
                    nc.scalar.activation(
                        pt, s_ps, mybir.ActivationFunctionType.Exp, scale=0.125
                    )
                    if ci >= 4 * s and ci < own_chunks:
                        nc.vector.tensor_mul(pt, pt, tri_sb[:, ci - 4 * s, :])
                    if ci >= own_chunks and (ci - own_chunks) // 4 == PADS[s] - 1:
                        nc.vector.tensor_scalar_mul(pt, pt, pm_sb[:, s:s + 1])
                    for r in range(4):
                        nc.tensor.matmul(
                            o_tiles[r], pt[:, r * 128:(r + 1) * 128], V1[:, g, :],
                            start=(ci == 0), stop=(ci == nC - 1),
                        )
                # ---- normalize into staging ----
                for r in range(4):
                    recip = work.tile([128, 1], f32, tag="recip")
                    nc.vector.reciprocal(recip, o_tiles[r][:, HEAD:HEAD + 1])
                    nc.vector.tensor_scalar_mul(
                        outs_sb[:, s * 4 + r, :], o_tiles[r][:, 0:HEAD], recip
                    )
            nc.sync.dma_start(out=out, in_=outs_sb)

    nc.compile()
    return nc


def _host_inputs(embedded, Wq, Wk, Wv):
    """Per-core input maps (host does layout only: transpose/permute/cast)."""
    bf = ml_dtypes.bfloat16
    emb = np.asarray(embedded, dtype=np.float32)
    wq = np.asarray(Wq, dtype=np.float32).astype(bf)
    wk = np.asarray(Wk, dtype=np.float32).astype(bf)
    wv = np.asarray(Wv, dtype=np.float32).astype(bf)

    # static triangular mask for the diagonal 512-block, [128, 4, 512]
    p = np.arange(128)[:, None, None]
    d = np.arange(4)[None, :, None]
    f = np.arange(BLK)[None, None, :]
    tri = ((d * 128 + p) <= f).astype(bf)

    in_maps = []
    for b in range(B):
        for role in range(2):
            order = OWN[role] + OWN[1 - role]
            xTb = emb[b].T  # [E, T]
            xTp = np.concatenate(
                [xTb[:, j * BLK:(j + 1) * BLK] for j in order], axis=1
            ).astype(bf)
            pm = np.broadcast_to(
                np.asarray(PADMASK[role], np.float32), (128, 4)
            ).astype(np.float32)
            in_maps.append({
                "xT": np.ascontiguousarray(xTp),
                "wq": wq, "wk": wk, "wv": wv,
                "tri": np.ascontiguousarray(tri),
                "padmask": np.ascontiguousarray(pm),
            })
    return in_maps


def _run(nc, in_maps, trace=False):
    from concourse.bass_utils import run_bass_kernel_spmd
    return run_bass_kernel_spmd(nc, in_maps, list(range(NCORES)), trace=trace)


def _assemble(results):
    head = np.empty((B, T, HEAD), dtype=np.float32)
    for core, r in enumerate(results):
        b, role = divmod(core, 2)
        o = np.asarray(r["out"])  # [128, 16, 64] partition-major
        o = o.transpose(1, 0, 2).reshape(16 * 128, HEAD)
        for s in range(4):
            j = OWN[role][s]
            head[b, j * BLK:(j + 1) * BLK, :] = o[s * BLK:(s + 1) * BLK, :]
    return np.tile(head, (1, 1, NH))


def kernel(embedded, Wq, Wk, Wv, num_heads):
    num_heads = int(num_heads)
    assert num_heads == NH

    if "nc" not in _prog_cache:
        _prog_cache["nc"] = _build_program()
    nc = _prog_cache["nc"]

    in_maps = _host_inputs(embedded, Wq, Wk, Wv)
    res = _run(nc, in_maps, trace=bool(int(os.environ.get("KERNEL_TRACE", "0"))))
    _prog_cache["last_result"] = res
    return _assemble(res.results)
